# revision 9
# baseline (speedup 1.0000x reference)
"""Trainium2 Bass kernel for nn_BLIPConceptPrefixModelV3 (topk_masking).

Math: reference's gather+softmax+mean collapses to per-token weights:
    h[b] = (1/C) * sum_s w[b,s] * qp[b,s,:],   w[b,s] = sum_c softmax16(qk[b,c,:])[s]
where softmax16 is softmax over the top-16 entries of each (b,c) row.
Top-16 selection runs in exp-space (all positive, so "remove" == "zero").

v13 layout (from 41.8us v12 baseline):
  * Hierarchical top-16 on the DVE: 6 per-chunk Max8 over 96-col slices
    produce 48 candidates; top-16 of the row == top-16 of the candidates
    unless one 96-chunk holds >8 of them (7/4096 rows on this data; the
    17th value then substitutes for the 16th — rel-err impact ~1e-4).
    This replaces three full 576-wide DVE scans with one, cutting the
    serial DVE stretch by ~1us/tile.
  * den = rowsum of the 16 top values (m8xy) instead of the stt
    accumulator; E is split into two 288-col tiles so each exp half
    un-gates its chunk Max8s without waiting for the other half.
  * The classifier is split per batch: b0's 12 cls matmuls run DURING
    the DVE stretch (real work replaces the v12 junk-echo block and
    keeps the PE HAM un-throttled), b1's run in the tail at full clock.
  * 1/C folded into the relu scale; wcol squash is a single plain copy.
  * cls_w packed nn-major and DMA'd in two halves so the py0 classifier
    data lands before the py1 data is needed; qn per-batch split kept.
  * All input DMA on ONE queue in strict need-order (concurrent queues
    round-robin per packet and starve the critical first megabyte).

Sharding: data-parallel over batch B=16 across 8 cores (2 batches/core),
weights replicated; no collectives.
"""

import os
import sys

sys.path.insert(0, "/opt/trn_rl_repo")

import numpy as np

B, S, D = 16, 577, 768
SP = S - 1  # 576 patch tokens
C, NCLS = 256, 1000
TOPK = 16
NCORES = 8
BPC = B // NCORES  # batches per core
ND = D // 128  # 6 d-chunks
NSC = 5  # s-chunks of 128 (last holds 64 real rows + 64 zero pad)
NCH = 6  # hierarchical top-k chunks per 576 row
CHW = SP // NCH  # 96
NWARM = int(os.environ.get("BLIP_NWARM", "7"))  # wide PE warm-up matmuls
NJA = int(os.environ.get("BLIP_NJA", "5"))  # junk pinned after exp(t1)
NJC = int(os.environ.get("BLIP_NJC", "7"))  # junk pinned after em(t3)

last_exec_time_ns = None
_cached = {}


def _apply_tile_patch():
    """walrus CoreV3 codegen rejects >2 sync-waits on a CTRL (Drain)
    instruction; split the TileContext tail-drain's waits across a chain of
    single-wait SP drains."""
    from concourse.tile import TileContext
    import concourse.mybir as mybir

    if getattr(TileContext, "_drain_patched", False):
        return

    MAX_WAITS = 1

    def _split_excess_waits(nc):
        """walrus rejects instructions carrying more than a couple of
        sync-waits; move the excess onto preceding same-engine Drain
        carriers (engines execute their stream in block order, so the
        waits still complete before the original instruction issues)."""
        for f in nc.m.functions:
            for blk in f.blocks:
                insts = list(blk.instructions)
                out = []
                changed = False
                for ins in insts:
                    si = getattr(ins, "sync_info", None)
                    eng = getattr(ins, "engine", None)
                    if si is not None and eng is not None and len(si.on_wait) > MAX_WAITS:
                        waits = list(si.on_wait)
                        si.on_wait.clear()
                        si.on_wait.extend(waits[:MAX_WAITS])
                        extra = waits[MAX_WAITS:]
                        for i in range(0, len(extra), MAX_WAITS):
                            carrier = mybir.InstDrain(
                                name=f"{ins.name}-w{i}",
                                ins=[],
                                outs=[],
                                engine=eng,
                            )
                            carrier.sync_info = mybir.SyncInfo(
                                on_wait=list(extra[i : i + MAX_WAITS]), on_update=[]
                            )
                            nc.register_instruction(carrier, overwrite=True)
                            out.append(carrier)
                        changed = True
                    out.append(ins)
                if changed:
                    blk.instructions.clear()
                    blk.instructions.extend(out)

    def _patched(self, tick_clock, wait_clock):
        import concourse.tile as tile_mod

        drain_inst = self.nc.sync.drain()
        wait_clock.add_sem_waits(
            drain_inst.ins, tile_mod.ScopedClock({None: tick_clock.global_clock})
        )
        waits = list(drain_inst.ins.sync_info.on_wait)
        if len(waits) > 1:
            drain_inst.ins.sync_info.on_wait.clear()
            drain_inst.ins.sync_info.on_wait.append(waits[0])
            for sw in waits[1:]:
                d = self.nc.sync.drain()
                if d.ins.sync_info is None:
                    d.ins.sync_info = mybir.SyncInfo(on_wait=[], on_update=[])
                d.ins.sync_info.on_wait.append(sw)

        self.nc.all_engine_barrier()
        assert self.sems is not None
        popped = self.nc._tile_sem_poison_stack.pop()
        assert popped is self._sem_poison
        self.nc.clear_and_free_semaphores(list(self.sems.allocated().values()))
        self.nc.all_engine_barrier()

        _split_excess_waits(self.nc)

    TileContext._drain_and_barrier = _patched
    TileContext._drain_patched = True


def _build_nc():
    import concourse.bass as bass
    import concourse.mybir as mybir
    from concourse.tile import TileContext

    f32 = mybir.dt.float32
    f16 = mybir.dt.float16
    Alu = mybir.AluOpType
    Act = mybir.ActivationFunctionType

    nc = bass.Bass()
    # host-packed, partition-major layouts
    qTd = nc.declare_dram_parameter("qTd", [128, BPC, ND, SP], f16, isOutput=False)
    qnd = nc.declare_dram_parameter("qnd", [128, BPC, NSC, D], f16, isOutput=False)
    cwd = nc.declare_dram_parameter("cwd", [128, 2, ND, 128], f16, isOutput=False)
    # nn-major: clsd[p, nn, dc, j] = cls_w[nn*500+j, dc*128+p]
    clsd = nc.declare_dram_parameter("clsd", [128, 2, ND, 500], f16, isOutput=False)
    y_d = nc.declare_dram_parameter("y", [BPC, NCLS], f32, isOutput=True)

    with TileContext(nc) as tc:
        with (
            tc.tile_pool(name="const", bufs=1) as constp,
            tc.tile_pool(name="qtp", bufs=1) as qtp,
            tc.tile_pool(name="ep", bufs=4) as ep,
            tc.tile_pool(name="wrkp", bufs=2) as wrkp,
            tc.tile_pool(name="emp", bufs=1) as emp,
            tc.tile_pool(name="smp", bufs=2) as smp,
        ):
            # ---- input DMA: ONE queue, strict FIFO need-order ----
            ones = constp.tile([128, 512], f16, tag="ones")
            nc.gpsimd.memset(ones[:], 1.0)

            cwt = constp.tile([128, 2, ND, 128], f16, tag="cw")
            nc.sync.dma_start(out=cwt[:], in_=cwd[:])
            qT0a = qtp.tile([128, 3, SP], f16, tag="qT0a")
            nc.sync.dma_start(out=qT0a[:, 0:1], in_=qTd[:, 0, 0:1])
            nc.sync.dma_start(out=qT0a[:, 1:3], in_=qTd[:, 0, 1:3])
            qT0b = qtp.tile([128, 3, SP], f16, tag="qT0b")
            nc.sync.dma_start(out=qT0b[:], in_=qTd[:, 0, 3:6])
            qT1 = qtp.tile([128, ND, SP], f16, tag="qT1")
            nc.sync.dma_start(out=qT1[:], in_=qTd[:, 1])
            qn01 = qtp.tile([128, BPC, NSC, D], f16, tag="qn01")
            nc.sync.dma_start(out=qn01[:, 0], in_=qnd[:, 0])
            clst = constp.tile([128, 2, ND, 500], f16, tag="cls")
            nc.sync.dma_start(out=clst[:, 0], in_=clsd[:, 0])
            nc.sync.dma_start(out=qn01[:, 1], in_=qnd[:, 1])
            nc.sync.dma_start(out=clst[:, 1], in_=clsd[:, 1])

            def cw_slice(dc, ct):
                return cwt[:, ct, dc, :]

            def qT_slice(b, dc, lo, hi):
                if b == 1:
                    return qT1[:, dc, lo:hi]
                t = qT0a if dc < 3 else qT0b
                return t[:, dc % 3, lo:hi]

            Em = {}
            R16 = {}
            chains = [(0, 0), (0, 1), (1, 0), (1, 1)]

            with (
                tc.tile_pool(name="qkp", bufs=2, space="PSUM") as qkp,
                tc.tile_pool(name="pwp", bufs=1, space="PSUM") as pwp,
                tc.tile_pool(name="htp", bufs=1, space="PSUM") as htp,
                tc.tile_pool(name="clsp", bufs=1, space="PSUM") as clsp,
            ):
                # ---- PE warm-up: un-throttle HAM during the DMA wait ----
                pwbig = pwp.tile([128, 12 + 480], f32, tag="pw", name="pwbig")
                # wide matmuls: the HAM up-clocks on sustained high MAC
                # utilization, which 1-col matmuls never provide
                for i in range(NWARM):
                    nc.tensor.matmul(
                        pwbig[:, 12:492],
                        lhsT=ones[:, 0:128],
                        rhs=ones[:, 0:480],
                        start=(i == 0),
                        stop=(i == NWARM - 1),
                    )

                # ---- phase 1: qk matmul (f16), exp, hierarchical top-16 ----
                for b, ct in chains:
                    qk0 = qkp.tile([128, 288], f32, tag="qk0")
                    qk1 = qkp.tile([128, 288], f32, tag="qk1")
                    qkh = (qk0, qk1)
                    # half-0 chain completes first so its exp (and the DVE
                    # chunk-max8s behind it) start while half-1 is running
                    for h in range(2):
                        for dc in range(ND):
                            nc.tensor.matmul(
                                qkh[h][:],
                                lhsT=cw_slice(dc, ct),
                                rhs=qT_slice(b, dc, h * 288, (h + 1) * 288),
                                start=(dc == 0),
                                stop=(dc == ND - 1),
                            )
                    E0 = ep.tile([128, 288], f16, tag="E0")
                    E1 = ep.tile([128, 288], f16, tag="E1")
                    if (b, ct) == (0, 0):
                        E1t1 = E1
                    nc.scalar.activation(E0[:], qk0[:], Act.Exp)
                    nc.scalar.activation(E1[:], qk1[:], Act.Exp)
                    Eh = (E0, E1)
                    # per-chunk top-8 candidates (exact top-16 of the row is
                    # among them unless a chunk holds >8 of the top-16)
                    cand = wrkp.tile([128, NCH, 8], f16, tag="cand")
                    for i in range(NCH):
                        hh = 0 if i < 3 else 1
                        nc.vector.max(
                            out=cand[:, i, :],
                            in_=Eh[hh][:, (i % 3) * CHW : (i % 3 + 1) * CHW],
                        )
                    m8xy = wrkp.tile([128, 16], f16, tag="m8xy")
                    nc.vector.max(out=m8xy[:, 0:8], in_=cand[:])
                    candz = wrkp.tile([128, NCH * 8], f16, tag="candz")
                    nc.vector.match_replace(
                        out=candz[:], in_to_replace=m8xy[:, 0:8],
                        in_values=cand[:], imm_value=0.0,
                    )
                    nc.vector.max(out=m8xy[:, 8:16], in_=candz[:])
                    # den = sum of the 16 surviving values; r16 = 1/den
                    den = smp.tile([128, 1], f32, tag=f"den{b}{ct}", bufs=1)
                    nc.vector.tensor_reduce(
                        out=den[:], in_=m8xy[:],
                        axis=mybir.AxisListType.X, op=Alu.add,
                    )
                    r16 = smp.tile([128, 1], f16, tag=f"r{b}{ct}", bufs=1)
                    with nc.allow_low_precision(reason="w-matmul runs fp16"):
                        nc.vector.reciprocal(r16[:], den[:])
                    R16[b, ct] = r16
                    # em = (E >= t16) * E, per half (t16 = 16th largest)
                    em = emp.tile([128, SP], f16, tag=f"em{b}{ct}")
                    nc.vector.scalar_tensor_tensor(
                        out=em[:, 0:288], in0=E0[:], scalar=m8xy[:, 15:16],
                        in1=E0[:], op0=Alu.is_ge, op1=Alu.mult,
                    )
                    nc.vector.scalar_tensor_tensor(
                        out=em[:, 288:576], in0=E1[:], scalar=m8xy[:, 15:16],
                        in1=E1[:], op0=Alu.is_ge, op1=Alu.mult,
                    )
                    Em[b, ct] = em

                # ---- junk block A: keep the PE HAM at 2.4GHz between the
                # qk chains and pw(b0). lhsT=E1(t1) pins it after exp(t1,h1)
                # so the scheduler cannot float it ahead of the qk chains. ----
                for i in range(NJA):
                    nc.tensor.matmul(
                        pwbig[:, 12:492],
                        lhsT=E1t1[:, 0:128],
                        rhs=ones[:, 0:480],
                        start=(i == 0),
                        stop=(i == NJA - 1),
                    )

                # ---- phase 2+3: w (pw/wcol), h, relu, classifier on PE ----
                py = [
                    clsp.tile([BPC, 500], f32, tag="py0", name="py0"),
                    clsp.tile([BPC, 500], f32, tag="py1", name="py1"),
                ]
                hTpA = htp.tile([128, 3, BPC], f32, tag="hTpA", name="hTpA")

                def ht_slice(dc, b):
                    if dc < 3:
                        return hTpA[0:128, dc, b : b + 1]
                    return pwbig[0:128, 6 + (dc - 3) * 2 + b : 7 + (dc - 3) * 2 + b]

                def ht_pair(dc):
                    if dc < 3:
                        return hTpA[:, dc, :]
                    return pwbig[:, 6 + (dc - 3) * 2 : 8 + (dc - 3) * 2]

                SCH = [(0, 128), (128, 128), (256, 128), (384, 128), (512, 64)]
                htrs = {}
                for b in range(BPC):
                    pw = pwbig
                    for sc, (s0, sz) in enumerate(SCH):
                        for ct in range(2):
                            nc.tensor.matmul(
                                pw[0:sz, sc : sc + 1],
                                lhsT=Em[b, ct][:, s0 : s0 + sz],
                                rhs=R16[b, ct][:],
                                start=(ct == 0),
                                stop=(ct == 1),
                            )
                    wcol = smp.tile([128, NSC], f16, tag=f"wc{b}", bufs=1)
                    # chunk 4 rows 64:128 are unwritten PSUM garbage; copied
                    # but never read (h matmuls slice [0:sz]).
                    nc.scalar.activation(wcol[:], pw[:, 0:NSC], Act.Copy)
                    for dc in range(ND):
                        for sc, (s0, sz) in enumerate(SCH):
                            nc.tensor.matmul(
                                ht_slice(dc, b),
                                lhsT=qn01[0:sz, b, sc, dc * 128 : (dc + 1) * 128],
                                rhs=wcol[0:sz, sc : sc + 1],
                                start=(sc == 0),
                                stop=(sc == NSC - 1),
                            )
                    if b == 0:
                        # junk block C: bridge the PE at high utilization
                        # through the back half of the DVE stretch until
                        # pw(b1) un-gates; lhsT=em(b1,ct0) pins it after
                        # the t3 stt so the scheduler cannot float it early.
                        for i in range(NJC):
                            nc.tensor.matmul(
                                pwbig[:, 12:492],
                                lhsT=Em[1, 0][:, 0:128],
                                rhs=ones[:, 0:480],
                                start=(i == 0),
                                stop=(i == NJC - 1),
                            )
                    else:
                        # tail: combined relu (both batches) + classifier
                        htrs2 = []
                        for dc in range(ND):
                            htr = smp.tile(
                                [128, BPC], f16, tag=f"hrt{dc}", bufs=1,
                                name=f"hrt{dc}",
                            )
                            nc.scalar.activation(
                                htr[:], ht_pair(dc), Act.Relu, scale=1.0 / C
                            )
                            htrs2.append(htr)
                        for dc in range(ND):
                            for nn in range(2):
                                nc.tensor.matmul(
                                    py[nn][:],
                                    lhsT=htrs2[dc][:],
                                    rhs=clst[:, nn, dc, :],
                                    start=(dc == 0),
                                    stop=(dc == ND - 1),
                                )
                ysb = smp.tile([BPC, NCLS], f32, tag="ysb", bufs=1)
                nc.scalar.activation(ysb[:, 0:500], py[0][:], Act.Copy)
                nc.sync.dma_start(out=y_d[:, 0:500], in_=ysb[:, 0:500])
                nc.vector.tensor_copy(out=ysb[:, 500:1000], in_=py[1][:])
                nc.gpsimd.dma_start(out=y_d[:, 500:1000], in_=ysb[:, 500:1000])
    return nc


def _register_ntff_hook():
    """The staged antenv package lacks axon_hooks; synthesize it and register
    the ctypes NTFF profile hook so trace=True yields exec_time_ns."""
    import types

    if "antenv.axon_hooks" in sys.modules:
        return
    try:
        import antenv
        from trn_agent_boot.trn_boot import _ntff_profile_via_ctypes

        mod = types.ModuleType("antenv.axon_hooks")
        _hook = [None]
        mod.set_axon_ntff_profile_hook = lambda h: _hook.__setitem__(0, h)
        mod.get_axon_ntff_profile_hook = lambda: _hook[0]
        sys.modules["antenv.axon_hooks"] = mod
        antenv.axon_hooks = mod
        mod.set_axon_ntff_profile_hook(
            _ntff_profile_via_ctypes("/opt/axon/libaxon_pjrt.so")
        )
    except Exception as e:  # profiling is best-effort
        print(f"ntff hook registration failed: {e}", file=sys.stderr)


def kernel(q, concept_w, cls_w, cls_b, topk):
    global last_exec_time_ns
    assert int(topk) == TOPK, f"kernel hardcodes top-k=16, got {topk}"

    _apply_tile_patch()
    if os.environ.get("BLIP_TRACE"):
        _register_ntff_hook()
    from concourse.bass_utils import run_bass_kernel_spmd

    if "nc" not in _cached:
        _cached["nc"] = _build_nc()
    nc = _cached["nc"]

    q = np.asarray(q, dtype=np.float32)
    qp = q[:, 1:, :].astype(np.float16)  # [B, 576, 768]
    # cwd[p, ct, dc, c'] = concept_w[ct*128+c', dc*128+p]
    cwd = np.ascontiguousarray(
        np.asarray(concept_w, dtype=np.float32).T.astype(np.float16)
        .reshape(ND, 128, 2, 128).transpose(1, 2, 0, 3)
    )
    # clsd[p, nn, dc, j] = cls_w[nn*500+j, dc*128+p]
    clsd = np.ascontiguousarray(
        np.asarray(cls_w, dtype=np.float32).T.astype(np.float16)
        .reshape(ND, 128, 2, 500).transpose(1, 2, 0, 3)
    )

    in_maps = []
    for core in range(NCORES):
        b0 = core * BPC
        qq = qp[b0 : b0 + BPC]  # [BPC, 576, 768]
        # qTd[p, b, dc, s] = qp[b, s, dc*128+p]
        qTd = np.ascontiguousarray(
            qq.transpose(2, 0, 1).reshape(ND, 128, BPC, SP).transpose(1, 2, 0, 3)
        )
        # qnd[p, b, sc, d] = qp[b, sc*128+p, d], zero-padded to 640 rows
        qq_pad = np.zeros((BPC, NSC * 128, D), dtype=np.float16)
        qq_pad[:, :SP] = qq
        qnd = np.ascontiguousarray(
            qq_pad.reshape(BPC, NSC, 128, D).transpose(2, 0, 1, 3)
        )
        in_maps.append({"qTd": qTd, "qnd": qnd, "cwd": cwd, "clsd": clsd})

    trace = bool(os.environ.get("BLIP_TRACE"))
    res = run_bass_kernel_spmd(nc, in_maps, list(range(NCORES)), trace=trace)
    last_exec_time_ns = res.exec_time_ns

    y = np.concatenate([res.results[i]["y"] for i in range(NCORES)], axis=0)
    y = y + np.asarray(cls_b, dtype=np.float32)[None, :]
    return np.ascontiguousarray(y, dtype=np.float32)


# revision 10
# speedup vs baseline: 1.0395x; 1.0395x over previous
"""Trainium2 Bass kernel for nn_BLIPConceptPrefixModelV3 (topk_masking).

Math: reference's gather+softmax+mean collapses to per-token weights:
    h[b] = (1/C) * sum_s w[b,s] * qp[b,s,:],   w[b,s] = sum_c softmax16(qk[b,c,:])[s]
where softmax16 is softmax over the top-16 entries of each (b,c) row.
Top-16 selection runs in exp-space (all positive, so "remove" == "zero").

v13 layout (from 41.8us v12 baseline):
  * Hierarchical top-16 on the DVE: 6 per-chunk Max8 over 96-col slices
    produce 48 candidates; top-16 of the row == top-16 of the candidates
    unless one 96-chunk holds >8 of them (7/4096 rows on this data; the
    17th value then substitutes for the 16th — rel-err impact ~1e-4).
    This replaces three full 576-wide DVE scans with one, cutting the
    serial DVE stretch by ~1us/tile.
  * den = rowsum of the 16 top values (m8xy) instead of the stt
    accumulator; E is split into two 288-col tiles so each exp half
    un-gates its chunk Max8s without waiting for the other half.
  * The classifier is split per batch: b0's 12 cls matmuls run DURING
    the DVE stretch (real work replaces the v12 junk-echo block and
    keeps the PE HAM un-throttled), b1's run in the tail at full clock.
  * 1/C folded into the relu scale; wcol squash is a single plain copy.
  * cls_w packed nn-major and DMA'd in two halves so the py0 classifier
    data lands before the py1 data is needed; qn per-batch split kept.
  * All input DMA on ONE queue in strict need-order (concurrent queues
    round-robin per packet and starve the critical first megabyte).

Sharding: data-parallel over batch B=16 across 8 cores (2 batches/core),
weights replicated; no collectives.
"""

import os
import sys

sys.path.insert(0, "/opt/trn_rl_repo")

import numpy as np

B, S, D = 16, 577, 768
SP = S - 1  # 576 patch tokens
C, NCLS = 256, 1000
TOPK = 16
NCORES = 8
BPC = B // NCORES  # batches per core
ND = D // 128  # 6 d-chunks
NSC = 5  # s-chunks of 128 (last holds 64 real rows + 64 zero pad)
NCH = 6  # hierarchical top-k chunks per 576 row
CHW = SP // NCH  # 96
NWARM = int(os.environ.get("BLIP_NWARM", "7"))  # wide PE warm-up matmuls
NJA = int(os.environ.get("BLIP_NJA", "5"))  # junk pinned after exp(t1)
NJC = int(os.environ.get("BLIP_NJC", "7"))  # junk pinned after em(t3)

last_exec_time_ns = None
_cached = {}


def _apply_tile_patch():
    """walrus CoreV3 codegen rejects >2 sync-waits on a CTRL (Drain)
    instruction; split the TileContext tail-drain's waits across a chain of
    single-wait SP drains."""
    from concourse.tile import TileContext
    import concourse.mybir as mybir

    if getattr(TileContext, "_drain_patched", False):
        return

    MAX_WAITS = 1

    def _split_excess_waits(nc):
        """walrus rejects instructions carrying more than a couple of
        sync-waits; move the excess onto preceding same-engine Drain
        carriers (engines execute their stream in block order, so the
        waits still complete before the original instruction issues)."""
        for f in nc.m.functions:
            for blk in f.blocks:
                insts = list(blk.instructions)
                out = []
                changed = False
                for ins in insts:
                    si = getattr(ins, "sync_info", None)
                    eng = getattr(ins, "engine", None)
                    if si is not None and eng is not None and len(si.on_wait) > MAX_WAITS:
                        waits = list(si.on_wait)
                        si.on_wait.clear()
                        si.on_wait.extend(waits[:MAX_WAITS])
                        extra = waits[MAX_WAITS:]
                        for i in range(0, len(extra), MAX_WAITS):
                            carrier = mybir.InstDrain(
                                name=f"{ins.name}-w{i}",
                                ins=[],
                                outs=[],
                                engine=eng,
                            )
                            carrier.sync_info = mybir.SyncInfo(
                                on_wait=list(extra[i : i + MAX_WAITS]), on_update=[]
                            )
                            nc.register_instruction(carrier, overwrite=True)
                            out.append(carrier)
                        changed = True
                    out.append(ins)
                if changed:
                    blk.instructions.clear()
                    blk.instructions.extend(out)

    def _patched(self, tick_clock, wait_clock):
        import concourse.tile as tile_mod

        drain_inst = self.nc.sync.drain()
        wait_clock.add_sem_waits(
            drain_inst.ins, tile_mod.ScopedClock({None: tick_clock.global_clock})
        )
        waits = list(drain_inst.ins.sync_info.on_wait)
        if len(waits) > 1:
            drain_inst.ins.sync_info.on_wait.clear()
            drain_inst.ins.sync_info.on_wait.append(waits[0])
            for sw in waits[1:]:
                d = self.nc.sync.drain()
                if d.ins.sync_info is None:
                    d.ins.sync_info = mybir.SyncInfo(on_wait=[], on_update=[])
                d.ins.sync_info.on_wait.append(sw)

        self.nc.all_engine_barrier()
        assert self.sems is not None
        popped = self.nc._tile_sem_poison_stack.pop()
        assert popped is self._sem_poison
        self.nc.clear_and_free_semaphores(list(self.sems.allocated().values()))
        self.nc.all_engine_barrier()

        _split_excess_waits(self.nc)

    TileContext._drain_and_barrier = _patched
    TileContext._drain_patched = True


def _build_nc():
    import concourse.bass as bass
    import concourse.mybir as mybir
    from concourse.tile import TileContext

    f32 = mybir.dt.float32
    f16 = mybir.dt.float16
    Alu = mybir.AluOpType
    Act = mybir.ActivationFunctionType

    nc = bass.Bass()
    # host-packed, partition-major layouts
    qTd = nc.declare_dram_parameter("qTd", [128, BPC, ND, SP], f16, isOutput=False)
    qnd = nc.declare_dram_parameter("qnd", [128, BPC, NSC, D], f16, isOutput=False)
    cwd = nc.declare_dram_parameter("cwd", [128, 2, ND, 128], f16, isOutput=False)
    # nn-major: clsd[p, nn, dc, j] = cls_w[nn*500+j, dc*128+p]
    clsd = nc.declare_dram_parameter("clsd", [128, 2, ND, 500], f16, isOutput=False)
    y_d = nc.declare_dram_parameter("y", [BPC, NCLS], f32, isOutput=True)

    with TileContext(nc) as tc:
        with (
            tc.tile_pool(name="const", bufs=1) as constp,
            tc.tile_pool(name="qtp", bufs=1) as qtp,
            tc.tile_pool(name="ep", bufs=4) as ep,
            tc.tile_pool(name="wrkp", bufs=2) as wrkp,
            tc.tile_pool(name="emp", bufs=1) as emp,
            tc.tile_pool(name="smp", bufs=2) as smp,
        ):
            # ---- input DMA: ONE queue, strict FIFO need-order ----
            ones = constp.tile([128, 512], f16, tag="ones")
            nc.gpsimd.memset(ones[:], 1.0)
            # padded classifier lhsT: relu writes cols 0:2 per dc; the other
            # 126 zero columns raise the matmul's MAC utilization 64x so the
            # tail classifier itself holds the PE HAM at 2.4GHz
            htrP = constp.tile([128, ND, 128], f16, tag="htrP")
            nc.gpsimd.memset(htrP[:], 0.0)

            cwt = constp.tile([128, 2, ND, 128], f16, tag="cw")
            nc.sync.dma_start(out=cwt[:], in_=cwd[:])
            qT0a = qtp.tile([128, 3, SP], f16, tag="qT0a")
            nc.sync.dma_start(out=qT0a[:, 0:1], in_=qTd[:, 0, 0:1])
            nc.sync.dma_start(out=qT0a[:, 1:3], in_=qTd[:, 0, 1:3])
            qT0b = qtp.tile([128, 3, SP], f16, tag="qT0b")
            nc.sync.dma_start(out=qT0b[:], in_=qTd[:, 0, 3:6])
            qT1 = qtp.tile([128, ND, SP], f16, tag="qT1")
            nc.sync.dma_start(out=qT1[:], in_=qTd[:, 1])
            qn01 = qtp.tile([128, BPC, NSC, D], f16, tag="qn01")
            nc.sync.dma_start(out=qn01[:, 0], in_=qnd[:, 0])
            clst = constp.tile([128, 2, ND, 500], f16, tag="cls")
            nc.sync.dma_start(out=clst[:, 0], in_=clsd[:, 0])
            nc.sync.dma_start(out=qn01[:, 1], in_=qnd[:, 1])
            nc.sync.dma_start(out=clst[:, 1], in_=clsd[:, 1])

            def cw_slice(dc, ct):
                return cwt[:, ct, dc, :]

            def qT_slice(b, dc, lo, hi):
                if b == 1:
                    return qT1[:, dc, lo:hi]
                t = qT0a if dc < 3 else qT0b
                return t[:, dc % 3, lo:hi]

            Em = {}
            R16 = {}
            chains = [(0, 0), (0, 1), (1, 0), (1, 1)]

            with (
                tc.tile_pool(name="qkp", bufs=2, space="PSUM") as qkp,
                tc.tile_pool(name="pwp", bufs=1, space="PSUM") as pwp,
                tc.tile_pool(name="htp", bufs=1, space="PSUM") as htp,
                tc.tile_pool(name="clsp", bufs=1, space="PSUM") as clsp,
            ):
                # ---- PE warm-up: un-throttle HAM during the DMA wait ----
                pwbig = pwp.tile([128, 12 + 480], f32, tag="pw", name="pwbig")
                # wide matmuls: the HAM up-clocks on sustained high MAC
                # utilization, which 1-col matmuls never provide
                for i in range(NWARM):
                    nc.tensor.matmul(
                        pwbig[:, 12:492],
                        lhsT=ones[:, 0:128],
                        rhs=ones[:, 0:480],
                        start=(i == 0),
                        stop=(i == NWARM - 1),
                    )

                # ---- phase 1: qk matmul (f16), exp, hierarchical top-16 ----
                for b, ct in chains:
                    qk0 = qkp.tile([128, 288], f32, tag="qk0")
                    qk1 = qkp.tile([128, 288], f32, tag="qk1")
                    qkh = (qk0, qk1)
                    # half-0 chain completes first so its exp (and the DVE
                    # chunk-max8s behind it) start while half-1 is running
                    for h in range(2):
                        for dc in range(ND):
                            nc.tensor.matmul(
                                qkh[h][:],
                                lhsT=cw_slice(dc, ct),
                                rhs=qT_slice(b, dc, h * 288, (h + 1) * 288),
                                start=(dc == 0),
                                stop=(dc == ND - 1),
                            )
                    E0 = ep.tile([128, 288], f16, tag="E0")
                    E1 = ep.tile([128, 288], f16, tag="E1")
                    if (b, ct) == (0, 0):
                        E1t1 = E1
                    nc.scalar.activation(E0[:], qk0[:], Act.Exp)
                    nc.scalar.activation(E1[:], qk1[:], Act.Exp)
                    Eh = (E0, E1)
                    # per-chunk top-8 candidates (exact top-16 of the row is
                    # among them unless a chunk holds >8 of the top-16)
                    cand = wrkp.tile([128, NCH, 8], f16, tag="cand")
                    for i in range(NCH):
                        hh = 0 if i < 3 else 1
                        nc.vector.max(
                            out=cand[:, i, :],
                            in_=Eh[hh][:, (i % 3) * CHW : (i % 3 + 1) * CHW],
                        )
                    m8xy = wrkp.tile([128, 16], f16, tag="m8xy")
                    nc.vector.max(out=m8xy[:, 0:8], in_=cand[:])
                    candz = wrkp.tile([128, NCH * 8], f16, tag="candz")
                    nc.vector.match_replace(
                        out=candz[:], in_to_replace=m8xy[:, 0:8],
                        in_values=cand[:], imm_value=0.0,
                    )
                    nc.vector.max(out=m8xy[:, 8:16], in_=candz[:])
                    # den = sum of the 16 surviving values; r16 = 1/den
                    den = smp.tile([128, 1], f32, tag=f"den{b}{ct}", bufs=1)
                    nc.vector.tensor_reduce(
                        out=den[:], in_=m8xy[:],
                        axis=mybir.AxisListType.X, op=Alu.add,
                    )
                    r16 = smp.tile([128, 1], f16, tag=f"r{b}{ct}", bufs=1)
                    with nc.allow_low_precision(reason="w-matmul runs fp16"):
                        nc.vector.reciprocal(r16[:], den[:])
                    R16[b, ct] = r16
                    # em = (E >= t16) * E, per half (t16 = 16th largest)
                    em = emp.tile([128, SP], f16, tag=f"em{b}{ct}")
                    nc.vector.scalar_tensor_tensor(
                        out=em[:, 0:288], in0=E0[:], scalar=m8xy[:, 15:16],
                        in1=E0[:], op0=Alu.is_ge, op1=Alu.mult,
                    )
                    nc.vector.scalar_tensor_tensor(
                        out=em[:, 288:576], in0=E1[:], scalar=m8xy[:, 15:16],
                        in1=E1[:], op0=Alu.is_ge, op1=Alu.mult,
                    )
                    Em[b, ct] = em

                # ---- junk block A: keep the PE HAM at 2.4GHz between the
                # qk chains and pw(b0). lhsT=E1(t1) pins it after exp(t1,h1)
                # so the scheduler cannot float it ahead of the qk chains. ----
                for i in range(NJA):
                    nc.tensor.matmul(
                        pwbig[:, 12:492],
                        lhsT=E1t1[:, 0:128],
                        rhs=ones[:, 0:480],
                        start=(i == 0),
                        stop=(i == NJA - 1),
                    )

                # ---- phase 2+3: w (pw/wcol), h, relu, classifier on PE ----
                py = [
                    clsp.tile([128, 500], f32, tag="py0", name="py0"),
                    clsp.tile([128, 500], f32, tag="py1", name="py1"),
                ]
                hTpA = htp.tile([128, 3, BPC], f32, tag="hTpA", name="hTpA")

                def ht_slice(dc, b):
                    if dc < 3:
                        return hTpA[0:128, dc, b : b + 1]
                    return pwbig[0:128, 6 + (dc - 3) * 2 + b : 7 + (dc - 3) * 2 + b]

                def ht_pair(dc):
                    if dc < 3:
                        return hTpA[:, dc, :]
                    return pwbig[:, 6 + (dc - 3) * 2 : 8 + (dc - 3) * 2]

                SCH = [(0, 128), (128, 128), (256, 128), (384, 128), (512, 64)]
                htrs = {}
                for b in range(BPC):
                    pw = pwbig
                    for sc, (s0, sz) in enumerate(SCH):
                        for ct in range(2):
                            nc.tensor.matmul(
                                pw[0:sz, sc : sc + 1],
                                lhsT=Em[b, ct][:, s0 : s0 + sz],
                                rhs=R16[b, ct][:],
                                start=(ct == 0),
                                stop=(ct == 1),
                            )
                    wcol = smp.tile([128, NSC], f16, tag=f"wc{b}", bufs=1)
                    # chunk 4 rows 64:128 are unwritten PSUM garbage; copied
                    # but never read (h matmuls slice [0:sz]).
                    if b == 0:
                        nc.scalar.activation(wcol[:], pw[:, 0:NSC], Act.Copy)
                    else:
                        nc.vector.tensor_copy(out=wcol[:], in_=pw[:, 0:NSC])
                    for dc in range(ND):
                        for sc, (s0, sz) in enumerate(SCH):
                            nc.tensor.matmul(
                                ht_slice(dc, b),
                                lhsT=qn01[0:sz, b, sc, dc * 128 : (dc + 1) * 128],
                                rhs=wcol[0:sz, sc : sc + 1],
                                start=(sc == 0),
                                stop=(sc == NSC - 1),
                            )
                    if b == 0:
                        # junk block C: bridge the PE at high utilization
                        # through the back half of the DVE stretch until
                        # pw(b1) un-gates; lhsT=em(b1,ct0) pins it after
                        # the t3 stt so the scheduler cannot float it early.
                        for i in range(NJC):
                            nc.tensor.matmul(
                                pwbig[:, 12:492],
                                lhsT=Em[1, 0][:, 0:128],
                                rhs=ones[:, 0:480],
                                start=(i == 0),
                                stop=(i == NJC - 1),
                            )
                    else:
                        # tail: combined relu (both batches) on the now-idle
                        # DVE, then the padded classifier
                        for dc in range(ND):
                            nc.vector.tensor_scalar(
                                out=htrP[:, dc, 0:BPC], in0=ht_pair(dc),
                                scalar1=0.0, scalar2=1.0 / C,
                                op0=Alu.max, op1=Alu.mult,
                            )
                        for dc in range(ND):
                            for nn in range(2):
                                nc.tensor.matmul(
                                    py[nn][:],
                                    lhsT=htrP[:, dc, :],
                                    rhs=clst[:, nn, dc, :],
                                    start=(dc == 0),
                                    stop=(dc == ND - 1),
                                )
                ysb = smp.tile([BPC, NCLS], f32, tag="ysb", bufs=1)
                nc.scalar.activation(ysb[:, 0:500], py[0][0:BPC, :], Act.Copy)
                nc.sync.dma_start(out=y_d[:, 0:500], in_=ysb[:, 0:500])
                nc.vector.tensor_copy(out=ysb[:, 500:1000], in_=py[1][0:BPC, :])
                nc.gpsimd.dma_start(out=y_d[:, 500:1000], in_=ysb[:, 500:1000])
    return nc


def _register_ntff_hook():
    """The staged antenv package lacks axon_hooks; synthesize it and register
    the ctypes NTFF profile hook so trace=True yields exec_time_ns."""
    import types

    if "antenv.axon_hooks" in sys.modules:
        return
    try:
        import antenv
        from trn_agent_boot.trn_boot import _ntff_profile_via_ctypes

        mod = types.ModuleType("antenv.axon_hooks")
        _hook = [None]
        mod.set_axon_ntff_profile_hook = lambda h: _hook.__setitem__(0, h)
        mod.get_axon_ntff_profile_hook = lambda: _hook[0]
        sys.modules["antenv.axon_hooks"] = mod
        antenv.axon_hooks = mod
        mod.set_axon_ntff_profile_hook(
            _ntff_profile_via_ctypes("/opt/axon/libaxon_pjrt.so")
        )
    except Exception as e:  # profiling is best-effort
        print(f"ntff hook registration failed: {e}", file=sys.stderr)


def kernel(q, concept_w, cls_w, cls_b, topk):
    global last_exec_time_ns
    assert int(topk) == TOPK, f"kernel hardcodes top-k=16, got {topk}"

    _apply_tile_patch()
    if os.environ.get("BLIP_TRACE"):
        _register_ntff_hook()
    from concourse.bass_utils import run_bass_kernel_spmd

    if "nc" not in _cached:
        _cached["nc"] = _build_nc()
    nc = _cached["nc"]

    q = np.asarray(q, dtype=np.float32)
    qp = q[:, 1:, :].astype(np.float16)  # [B, 576, 768]
    # cwd[p, ct, dc, c'] = concept_w[ct*128+c', dc*128+p]
    cwd = np.ascontiguousarray(
        np.asarray(concept_w, dtype=np.float32).T.astype(np.float16)
        .reshape(ND, 128, 2, 128).transpose(1, 2, 0, 3)
    )
    # clsd[p, nn, dc, j] = cls_w[nn*500+j, dc*128+p]
    clsd = np.ascontiguousarray(
        np.asarray(cls_w, dtype=np.float32).T.astype(np.float16)
        .reshape(ND, 128, 2, 500).transpose(1, 2, 0, 3)
    )

    in_maps = []
    for core in range(NCORES):
        b0 = core * BPC
        qq = qp[b0 : b0 + BPC]  # [BPC, 576, 768]
        # qTd[p, b, dc, s] = qp[b, s, dc*128+p]
        qTd = np.ascontiguousarray(
            qq.transpose(2, 0, 1).reshape(ND, 128, BPC, SP).transpose(1, 2, 0, 3)
        )
        # qnd[p, b, sc, d] = qp[b, sc*128+p, d], zero-padded to 640 rows
        qq_pad = np.zeros((BPC, NSC * 128, D), dtype=np.float16)
        qq_pad[:, :SP] = qq
        qnd = np.ascontiguousarray(
            qq_pad.reshape(BPC, NSC, 128, D).transpose(2, 0, 1, 3)
        )
        in_maps.append({"qTd": qTd, "qnd": qnd, "cwd": cwd, "clsd": clsd})

    trace = bool(os.environ.get("BLIP_TRACE"))
    res = run_bass_kernel_spmd(nc, in_maps, list(range(NCORES)), trace=trace)
    last_exec_time_ns = res.exec_time_ns

    y = np.concatenate([res.results[i]["y"] for i in range(NCORES)], axis=0)
    y = y + np.asarray(cls_b, dtype=np.float32)[None, :]
    return np.ascontiguousarray(y, dtype=np.float32)
